# revision 3
# baseline (speedup 1.0000x reference)
"""Trainium2 Bass kernel for ContrastiveAttention.

Data-parallel over batch B=512 across 8 NeuronCores (64 batches/core).
All weights + the 100-row normality pool are replicated per core (shipped
pre-transposed from host so every matmul reads operands in its natural
[K-on-partitions] layout). Per core:

  Preamble (PE + small DVE/ACT ops):
    v_hat^T = Wproj^T-chunks @ fc^T            [4x[128,64] chunks]
    q_all[r]^T = agg^T @ v_hat^T               [3x4 chunks]
    scores[r] = q_all[r]^T.T @ pool^T          [64,100] + softmax
    v_agg^T = pool-chunks @ softmax(w)^T / 3   [4 chunks]
    q,k,v = v_hat/v_agg @ W^T + b              [64,512] psums
    sim gate, v_common^T via diag(sigmoid) matmul,
    v_diff = relu((v_hat - v_common) @ cw^T)   [64,512]
    vds2 [128,512] = res_scale*v_diff doubled across partition halves
    bias2 [128,1]  = v_diff@gw_diff + gb doubled

  Streaming (DMA-bound, 51.4MB/core att in+out):
    tiles [128, 7, 512]: partitions = 2 patch-blocks x 64 batches
    s = bias + att . gw_att      (fused tensor_tensor_reduce per patch)
    g = sigmoid(s)               (ACT)
    out = att + g * vds2         (fused scalar_tensor_tensor per patch)
"""
import numpy as np

D = 512
DFC = 2048
PL = 100
R = 3
B = 512
N = 196
NCORES = 8
BL = B // NCORES  # 64 batches per core
FP = 7            # patches per partition-block
NT = N // (2 * FP)  # 14 streaming tiles per core

_CACHE = {}


def _build():
    import concourse.bacc as bacc
    import concourse.tile as tile
    from concourse import mybir

    f32 = mybir.dt.float32
    AF = mybir.ActivationFunctionType
    OP = mybir.AluOpType
    AX = mybir.AxisListType
    scale = 1.0 / float(np.sqrt(D))

    nc = bacc.Bacc("TRN2", target_bir_lowering=False, debug=False)

    att = nc.dram_tensor("att", [BL, N, D], f32, kind="ExternalInput")
    fcT = nc.dram_tensor("fcT", [DFC, BL], f32, kind="ExternalInput")
    wprojT = nc.dram_tensor("wprojT", [DFC, D], f32, kind="ExternalInput")
    aggT = nc.dram_tensor("aggT", [R, D, D], f32, kind="ExternalInput")
    poolT = nc.dram_tensor("poolT", [D, PL], f32, kind="ExternalInput")
    poolm = nc.dram_tensor("poolm", [PL, D], f32, kind="ExternalInput")
    qwT = nc.dram_tensor("qwT", [D, D], f32, kind="ExternalInput")
    kwT = nc.dram_tensor("kwT", [D, D], f32, kind="ExternalInput")
    vwT = nc.dram_tensor("vwT", [D, D], f32, kind="ExternalInput")
    cwT = nc.dram_tensor("cwT", [D, D], f32, kind="ExternalInput")
    qb = nc.dram_tensor("qb", [1, D], f32, kind="ExternalInput")
    kb = nc.dram_tensor("kb", [1, D], f32, kind="ExternalInput")
    vb = nc.dram_tensor("vb", [1, D], f32, kind="ExternalInput")
    cb = nc.dram_tensor("cb", [1, D], f32, kind="ExternalInput")
    gwa_b = nc.dram_tensor("gwa_b", [128, D], f32, kind="ExternalInput")
    gwd_b = nc.dram_tensor("gwd_b", [128, D], f32, kind="ExternalInput")
    res_b = nc.dram_tensor("res_b", [128, 1], f32, kind="ExternalInput")
    gb11 = nc.dram_tensor("gb11", [1, 1], f32, kind="ExternalInput")
    ident = nc.dram_tensor("ident", [128, 128], f32, kind="ExternalInput")
    idd2 = nc.dram_tensor("idd2", [BL, 128], f32, kind="ExternalInput")
    ones128 = nc.dram_tensor("ones128", [1, 128], f32, kind="ExternalInput")
    out_d = nc.dram_tensor("out", [BL, N, D], f32, kind="ExternalOutput")

    with tile.TileContext(nc) as tc:
        with (
            tc.tile_pool(name="persist", bufs=1) as persist,
            tc.tile_pool(name="stream_att", bufs=2) as apool,
            tc.tile_pool(name="stream_out", bufs=2) as opool,
            tc.tile_pool(name="stream_scr", bufs=2) as scrpool,
            tc.tile_pool(name="stream_g", bufs=3) as gpool,
        ):
            # ---------------- preamble ----------------
            with (
                tc.tile_pool(name="weights", bufs=1) as wpool,
                tc.tile_pool(name="pre_sb", bufs=1) as spool,
                tc.tile_pool(name="pre_ps", bufs=4, space="PSUM") as ppool,
            ):
                wproj_t = wpool.tile([128, DFC // 128, D], f32)
                nc.sync.dma_start(
                    out=wproj_t[:],
                    in_=wprojT.ap().rearrange("(k p) d -> p k d", p=128),
                )
                fc_t = wpool.tile([128, DFC // 128, BL], f32)
                nc.sync.dma_start(
                    out=fc_t[:], in_=fcT.ap().rearrange("(k p) b -> p k b", p=128)
                )
                agg_t = wpool.tile([128, R, 4, D], f32)
                nc.sync.dma_start(
                    out=agg_t[:],
                    in_=aggT.ap().rearrange("r (k p) o -> p r k o", p=128),
                )
                poolT_t = wpool.tile([128, 4, PL], f32)
                nc.sync.dma_start(
                    out=poolT_t[:], in_=poolT.ap().rearrange("(c p) q -> p c q", p=128)
                )
                pool_t = wpool.tile([PL, D], f32)
                nc.sync.dma_start(out=pool_t[:], in_=poolm.ap())
                qw_t = wpool.tile([128, 4, D], f32)
                nc.sync.dma_start(
                    out=qw_t[:], in_=qwT.ap().rearrange("(k p) o -> p k o", p=128)
                )
                kw_t = wpool.tile([128, 4, D], f32)
                nc.sync.dma_start(
                    out=kw_t[:], in_=kwT.ap().rearrange("(k p) o -> p k o", p=128)
                )
                vw_t = wpool.tile([128, 4, D], f32)
                nc.sync.dma_start(
                    out=vw_t[:], in_=vwT.ap().rearrange("(k p) o -> p k o", p=128)
                )
                cw_t = wpool.tile([128, 4, D], f32)
                nc.sync.dma_start(
                    out=cw_t[:], in_=cwT.ap().rearrange("(k p) o -> p k o", p=128)
                )
                qb_t = wpool.tile([1, D], f32)
                nc.sync.dma_start(out=qb_t[:], in_=qb.ap())
                kb_t = wpool.tile([1, D], f32)
                nc.sync.dma_start(out=kb_t[:], in_=kb.ap())
                vb_t = wpool.tile([1, D], f32)
                nc.sync.dma_start(out=vb_t[:], in_=vb.ap())
                cb_t = wpool.tile([1, D], f32)
                nc.sync.dma_start(out=cb_t[:], in_=cb.ap())
                gwd_t = wpool.tile([128, D], f32)
                nc.sync.dma_start(out=gwd_t[:], in_=gwd_b.ap())
                res_t = wpool.tile([128, 1], f32)
                nc.sync.dma_start(out=res_t[:], in_=res_b.ap())
                gb_t = wpool.tile([1, 1], f32)
                nc.sync.dma_start(out=gb_t[:], in_=gb11.ap())
                ident_t = wpool.tile([128, 128], f32)
                nc.sync.dma_start(out=ident_t[:], in_=ident.ap())
                idd2_t = wpool.tile([BL, 128], f32)
                nc.sync.dma_start(out=idd2_t[:], in_=idd2.ap())
                ones_t = wpool.tile([1, 128], f32)
                nc.sync.dma_start(out=ones_t[:], in_=ones128.ap())

                # v_hat^T [128, 4, BL]
                vhT = spool.tile([128, 4, BL], f32)
                for c in range(4):
                    ps = ppool.tile([128, BL], f32, tag="ps")
                    for k in range(DFC // 128):
                        nc.tensor.matmul(
                            ps[:],
                            wproj_t[:, k, c * 128 : (c + 1) * 128],
                            fc_t[:, k, :],
                            start=(k == 0),
                            stop=(k == DFC // 128 - 1),
                        )
                    nc.scalar.mul(vhT[:, c, :], ps[:], 1.0)

                # q_all^T [128, R, 4, BL]
                qaT = spool.tile([128, R, 4, BL], f32)
                for r in range(R):
                    for c in range(4):
                        ps = ppool.tile([128, BL], f32, tag="ps")
                        for k in range(4):
                            nc.tensor.matmul(
                                ps[:],
                                agg_t[:, r, k, c * 128 : (c + 1) * 128],
                                vhT[:, k, :],
                                start=(k == 0),
                                stop=(k == 3),
                            )
                        nc.scalar.mul(qaT[:, r, c, :], ps[:], 1.0)

                # scores -> softmax -> transposed weights wTs [PL, R, BL]
                wTs = spool.tile([PL, R, BL], f32)
                for r in range(R):
                    ps_sc = ppool.tile([BL, PL], f32, tag="ps")
                    for c in range(4):
                        nc.tensor.matmul(
                            ps_sc[:],
                            qaT[:, r, c, :],
                            poolT_t[:, c, :],
                            start=(c == 0),
                            stop=(c == 3),
                        )
                    mx = spool.tile([BL, 1], f32, tag="mx")
                    nc.vector.reduce_max(mx[:], ps_sc[:], axis=AX.X)
                    nmx = spool.tile([BL, 1], f32, tag="nmx")
                    nc.vector.tensor_scalar_mul(nmx[:], mx[:], -scale)
                    e = spool.tile([BL, PL], f32, tag="e")
                    ssum = spool.tile([BL, 1], f32, tag="ssum")
                    nc.scalar.activation(
                        out=e[:],
                        in_=ps_sc[:],
                        func=AF.Exp,
                        bias=nmx[:],
                        scale=scale,
                        accum_out=ssum[:],
                    )
                    rec = spool.tile([BL, 1], f32, tag="rec")
                    nc.vector.reciprocal(rec[:], ssum[:])
                    wn = spool.tile([BL, PL], f32, tag="wn")
                    nc.vector.tensor_scalar_mul(wn[:], e[:], rec[:])
                    ps_wT = ppool.tile([PL, BL], f32, tag="ps")
                    nc.tensor.transpose(ps_wT[:], wn[:], ident_t[:BL, :BL])
                    nc.scalar.mul(wTs[:, r, :], ps_wT[:], 1.0)

                # v_agg^T [128, 4, BL] (divide by ROUNDS during psum->sbuf copy)
                vaT = spool.tile([128, 4, BL], f32)
                for c in range(4):
                    ps = ppool.tile([128, BL], f32, tag="ps")
                    for r in range(R):
                        nc.tensor.matmul(
                            ps[:],
                            pool_t[:, c * 128 : (c + 1) * 128],
                            wTs[:, r, :],
                            start=(r == 0),
                            stop=(r == R - 1),
                        )
                    nc.scalar.mul(vaT[:, c, :], ps[:], 1.0 / R)

                # q, k, v [BL, D] psums
                ps_q = ppool.tile([BL, D], f32, tag="ps")
                for k in range(4):
                    nc.tensor.matmul(
                        ps_q[:], vhT[:, k, :], qw_t[:, k, :],
                        start=(k == 0), stop=False,
                    )
                nc.tensor.matmul(
                    ps_q[:], ones_t[:1, :BL], qb_t[:], start=False, stop=True
                )
                ps_k = ppool.tile([BL, D], f32, tag="ps")
                for k in range(4):
                    nc.tensor.matmul(
                        ps_k[:], vaT[:, k, :], kw_t[:, k, :],
                        start=(k == 0), stop=False,
                    )
                nc.tensor.matmul(
                    ps_k[:], ones_t[:1, :BL], kb_t[:], start=False, stop=True
                )
                ps_v = ppool.tile([BL, D], f32, tag="ps")
                for k in range(4):
                    nc.tensor.matmul(
                        ps_v[:], vaT[:, k, :], vw_t[:, k, :],
                        start=(k == 0), stop=False,
                    )
                nc.tensor.matmul(
                    ps_v[:], ones_t[:1, :BL], vb_t[:], start=False, stop=True
                )

                # sim = sum(q*k) -> sigmoid gate
                q_sb = spool.tile([BL, D], f32)
                nc.scalar.mul(q_sb[:], ps_q[:], 1.0)
                scr_qk = spool.tile([BL, D], f32)
                simt = spool.tile([BL, 1], f32)
                nc.vector.scalar_tensor_tensor(
                    out=scr_qk[:],
                    in0=q_sb[:],
                    scalar=1.0,
                    in1=ps_k[:],
                    op0=OP.mult,
                    op1=OP.mult,
                    accum_out=simt[:],
                )
                sg = spool.tile([BL, 1], f32)
                nc.scalar.activation(
                    out=sg[:], in_=simt[:], func=AF.Sigmoid, scale=scale
                )

                # v_common^T chunks via diag(sg) matmul; dT = vhT - vcT
                v_sb = spool.tile([BL, D], f32)
                nc.scalar.mul(v_sb[:], ps_v[:], 1.0)
                diag = spool.tile([BL, BL], f32)
                nc.vector.tensor_scalar_mul(diag[:], ident_t[:BL, :BL], sg[:])
                ps_vcT = ppool.tile([128, 4, BL], f32, tag="ps")
                for c in range(4):
                    nc.tensor.matmul(
                        ps_vcT[:, c, :],
                        v_sb[:, c * 128 : (c + 1) * 128],
                        diag[:],
                        start=True,
                        stop=True,
                    )
                dT = spool.tile([128, 4, BL], f32)
                nc.vector.tensor_sub(dT[:], vhT[:], ps_vcT[:])

                # v_diff = relu(dT.T @ cw^T + cb)
                ps_vd = ppool.tile([BL, D], f32, tag="ps")
                for k in range(4):
                    nc.tensor.matmul(
                        ps_vd[:], dT[:, k, :], cw_t[:, k, :],
                        start=(k == 0), stop=False,
                    )
                nc.tensor.matmul(
                    ps_vd[:], ones_t[:1, :BL], cb_t[:], start=False, stop=True
                )
                vdiff = spool.tile([BL, D], f32)
                nc.scalar.activation(out=vdiff[:], in_=ps_vd[:], func=AF.Relu)

                # vds2 [128, D] = res * v_diff doubled across partition halves
                dstack = spool.tile([BL, 128], f32)
                nc.vector.tensor_scalar_mul(dstack[:], idd2_t[:], res_t[:BL, :])
                ps_vds2 = ppool.tile([128, D], f32, tag="ps")
                nc.tensor.matmul(
                    ps_vds2[:], dstack[:], vdiff[:], start=True, stop=True
                )
                vds2 = persist.tile([128, D], f32)
                nc.scalar.mul(vds2[:], ps_vds2[:], 1.0)

                # bias2 [128, 1] = (v_diff @ gw_diff + gb) doubled
                scr_d = spool.tile([BL, D], f32)
                dv = spool.tile([BL, 1], f32)
                nc.vector.scalar_tensor_tensor(
                    out=scr_d[:],
                    in0=vdiff[:],
                    scalar=1.0,
                    in1=gwd_t[:BL, :],
                    op0=OP.mult,
                    op1=OP.mult,
                    accum_out=dv[:],
                )
                ps_b2 = ppool.tile([128, 1], f32, tag="ps")
                nc.tensor.matmul(ps_b2[:], idd2_t[:], dv[:], start=True, stop=False)
                nc.tensor.matmul(
                    ps_b2[:], ones_t[:], gb_t[:], start=False, stop=True
                )
                bias2 = persist.tile([128, 1], f32)
                nc.scalar.mul(bias2[:], ps_b2[:], 1.0)

            # gw_att broadcast stays resident for the whole stream
            gwa_t = persist.tile([128, D], f32)
            nc.sync.dma_start(out=gwa_t[:], in_=gwa_b.ap())

            # ---------------- streaming ----------------
            att_ap = att.ap()
            out_ap = out_d.ap()
            for it in range(NT):
                n0 = it * 2 * FP
                att_t = apool.tile([128, FP, D], f32)
                for t in range(2):
                    nc.sync.dma_start(
                        out=att_t[t * BL : (t + 1) * BL, :, :],
                        in_=att_ap[:, n0 + t * FP : n0 + (t + 1) * FP, :],
                    )
                s_t = gpool.tile([128, FP], f32, tag="s_t")
                for j in range(FP):
                    scr = scrpool.tile([128, D], f32)
                    nc.vector.scalar_tensor_tensor(
                        out=scr[:],
                        in0=att_t[:, j, :],
                        scalar=1.0,
                        in1=gwa_t[:],
                        op0=OP.bypass,
                        op1=OP.mult,
                        accum_out=s_t[:, j : j + 1],
                    )
                g_t = gpool.tile([128, FP], f32, tag="g_t")
                nc.scalar.activation(
                    out=g_t[:], in_=s_t[:], func=AF.Sigmoid, bias=bias2[:]
                )
                out_t = opool.tile([128, FP, D], f32)
                for j in range(FP):
                    nc.vector.scalar_tensor_tensor(
                        out=out_t[:, j, :],
                        in0=vds2[:],
                        scalar=g_t[:, j : j + 1],
                        in1=att_t[:, j, :],
                        op0=OP.mult,
                        op1=OP.add,
                    )
                for t in range(2):
                    nc.sync.dma_start(
                        out=out_ap[:, n0 + t * FP : n0 + (t + 1) * FP, :],
                        in_=out_t[t * BL : (t + 1) * BL, :, :],
                    )

    nc.compile()
    return nc


def _get_nc():
    if "nc" not in _CACHE:
        _CACHE["nc"] = _build()
    return _CACHE["nc"]


def kernel(
    att_feats, fc_feats, pool, fc_proj_w, agg_w, qw, qb, kw, kb, vw, vb,
    cw, cb, gw, gb, res_scale,
):
    from concourse.bass_utils import run_bass_kernel_spmd

    f = np.float32
    att_feats = np.asarray(att_feats, f)
    fc_feats = np.asarray(fc_feats, f)
    shared = {
        "wprojT": np.ascontiguousarray(np.asarray(fc_proj_w, f).T),
        "aggT": np.ascontiguousarray(np.asarray(agg_w, f).transpose(0, 2, 1)),
        "poolT": np.ascontiguousarray(np.asarray(pool, f).T),
        "poolm": np.ascontiguousarray(np.asarray(pool, f)),
        "qwT": np.ascontiguousarray(np.asarray(qw, f).T),
        "kwT": np.ascontiguousarray(np.asarray(kw, f).T),
        "vwT": np.ascontiguousarray(np.asarray(vw, f).T),
        "cwT": np.ascontiguousarray(np.asarray(cw, f).T),
        "qb": np.asarray(qb, f).reshape(1, D),
        "kb": np.asarray(kb, f).reshape(1, D),
        "vb": np.asarray(vb, f).reshape(1, D),
        "cb": np.asarray(cb, f).reshape(1, D),
        "gwa_b": np.ascontiguousarray(
            np.broadcast_to(np.asarray(gw, f)[:1, :D], (128, D))
        ),
        "gwd_b": np.ascontiguousarray(
            np.broadcast_to(np.asarray(gw, f)[:1, D:], (128, D))
        ),
        "res_b": np.full((128, 1), np.asarray(res_scale, f)[0], f),
        "gb11": np.asarray(gb, f).reshape(1, 1),
        "ident": np.eye(128, dtype=f),
        "idd2": np.concatenate([np.eye(BL, dtype=f), np.eye(BL, dtype=f)], axis=1),
        "ones128": np.ones((1, 128), f),
    }
    in_maps = []
    for i in range(NCORES):
        m = dict(shared)
        m["att"] = np.ascontiguousarray(att_feats[i * BL : (i + 1) * BL])
        m["fcT"] = np.ascontiguousarray(fc_feats[i * BL : (i + 1) * BL].T)
        in_maps.append(m)

    nc = _get_nc()
    res = run_bass_kernel_spmd(nc, in_maps, list(range(NCORES)))
    out = np.concatenate(
        [res.results[i]["out"] for i in range(NCORES)], axis=0
    )
    return np.ascontiguousarray(out, f)
